# revision 35
# baseline (speedup 1.0000x reference)
"""Trainium2 Bass kernel for nn_CGAMotorModel.

Reference computes, for B=512, H=1024, D=5 multivector channels of Cl(4,1):
    W_x[b,h]  = sum_d x[b,d] o W_in[h,d]          (o = geometric product)
    h_free    = (1 - (1-dt)^n) * W_x              (closed form of the scan)
    out[b]    = sum_h h_free[b,h] o W_out[h]

By associativity/bilinearity of the geometric product this collapses to
    out[b] = c * sum_d x[b,d] o K_d,   K_d = sum_h W_in[h,d] o W_out[h]
with c = 1 - 0.9^10.

H-tensor-parallel across 8 cores: each core takes a 128-row H-chunk, builds a
partial M and multiplies the FULL batch by it; the host sums the 8 partial
outputs (M is linear in the per-chunk S).

Per-core device program (all matmul operands bf16, PSUM accumulation f32):
    S^T[r,(d,q)] = sum_h W_out[h,r] * W_in[h,(d,q)]      (1 matmul, K=128)
    K^T[r',d]    = c * sum_q C[q,:,:] slab.T @ S_q^T     (32 matmuls, K=32)
    M'[d,(p,m)]  = sum_r' K^T[r',d] * CT[r',(p,m)]       (3 matmuls; chunked
                   so the PSUM->SBUF copies pipeline across DVE and ACT)
    M            = one-hot-matmul repack of M' into two 80-row group tiles
                   (p<16 / p>=16; within a group row = 16d+q, 64+q with
                   q = p%16). Each of the 32 matmuls (K=5) scatters ALL
                   FIVE d-rows of one p — partition regrouping on the PE,
                   which previously needed a DRAM bounce.
    out[b,m]     = sum_groups X^T_g[.,b] * M_g[.,m]      (8 matmuls, 4 blocks
                   of K=80+80; one group's halves open each block while the
                   other group's copy lands, then its halves close them)
X^T arrives pre-packed from the host in the same 80-row group layout, so no
on-device transposes are needed. CT[r, p*32+m] = C[p, r, m] serves both the
K-step (as per-q slabs) and the M'-step; CTK carries the free-phase
geometric-series constant. The one-hot lhsT slabs are sliding windows into a
tiny [5, 95] table shared by both groups (DMA issue cost scales with
bytes-per-partition). PSUM banks are packed (kpsum and two M' chunks ride in
spare bank space) so all four opsum blocks get their own bank and the finals
free-run.
"""

import numpy as np
from ml_dtypes import bfloat16

import concourse.bass as bass
import concourse.mybir as mybir
import concourse.tile as tile
from concourse import bacc
from concourse.bass_utils import run_bass_kernel_spmd

B, H, D, MV = 512, 1024, 5, 32
N_CORES = 8
DT, N_FREE = 0.1, 10
C_SCALE = 1.0 - (1.0 - DT) ** N_FREE
F32 = mybir.dt.float32
BF16 = mybir.dt.bfloat16

# X^T / M row split: two 80-row groups, one per p-half. Within a group,
# row = 16d + (p%16) for d<4 and 64 + (p%16) for d=4.
RG = 80


def _cayley_np() -> np.ndarray:
    """Cayley table for Cl(4,1), metric diag(1,1,1,1,-1). C[a,b,a^b] = sign."""
    metric = np.array([1.0, 1.0, 1.0, 1.0, -1.0], dtype=np.float32)
    C = np.zeros((32, 32, 32), dtype=np.float32)
    for a in range(32):
        for b in range(32):
            cnt = 0
            aa = a >> 1
            while aa:
                cnt += bin(aa & b).count("1")
                aa >>= 1
            s = -1.0 if (cnt & 1) else 1.0
            common = a & b
            for i in range(5):
                if (common >> i) & 1:
                    s *= metric[i]
            C[a, b, a ^ b] = s
    return C


# CT[r, q*32 + r'] = C[q, r, r'].
CT = np.ascontiguousarray(_cayley_np().transpose(1, 0, 2)).reshape(32, 1024)
CTK = (C_SCALE * CT).astype(np.float32)
CT_BF = CT.astype(bfloat16)
CTK_BF = CTK.astype(bfloat16)


def _onehot_tables() -> np.ndarray:
    """Sliding-window one-hot table [5, 95] (DMA issue cost scales with
    bytes-per-partition, so explicit per-p tables are replaced by windows).
    E[d, j] = 1 iff j == 16d+15 (d<4) or (d==4 and j==79). The p-slab
    E[:, 15-q : 95-q] (q = p%16) is then [d, i] = 1 iff i == 16d+q
    (d<4) or i == 64+q (d==4) — scattering all five M'[d, 32p+m] rows of
    one p into the 80-row group tile in a single matmul. Both p-halves
    share the same windows."""
    e = np.zeros((D, 95), dtype=np.float32)
    for d in range(4):
        e[d, 16 * d + 15] = 1.0
    e[4, 79] = 1.0
    return e


ECAT_BF = _onehot_tables().astype(bfloat16)


def build_program(
    w_eng: str = "sync",
    ct_eng: str = "gpsimd",
    x1_eng: str = "scalar",
    x2_eng: str = "gpsimd",
    e_eng: str = "sync",
    out_eng: str = "sync",
) -> bass.Bass:
    # Bacc (not plain Bass): its compile pass moves multi-sem matmul waits
    # onto LdWeights — walrus rejects Matmult with >1 sync wait otherwise.
    nc = bacc.Bacc()
    # wcat = [W_in.reshape(128,160) | W_out.reshape(128,32)] for this H-chunk
    wcat = nc.dram_tensor("wcat", [128, 192], BF16, kind="ExternalInput")
    ct = nc.dram_tensor("ct", [32, 1024], BF16, kind="ExternalInput")
    ctk = nc.dram_tensor("ctk", [32, 1024], BF16, kind="ExternalInput")
    ecat = nc.dram_tensor("ecat", [D, 95], BF16, kind="ExternalInput")
    # xt rows: group A (p<16) then group B (p>=16), each in the 80-row
    # (16d+q, 64+q) packing that matches the M tiles
    xt = nc.dram_tensor("xt", [2 * RG, B], BF16, kind="ExternalInput")
    # native osb layout [b_in_block, (t m)] — host de-interleaves; a flat
    # 64KB write avoids the sub-512B-contiguity 2x DMA penalty
    out = nc.dram_tensor("out", [128, 4 * MV], F32, kind="ExternalOutput")

    with tile.TileContext(nc) as tc:
        with (
            tc.tile_pool(name="sb", bufs=1) as sb,
            tc.tile_pool(name="ps", bufs=1, space="PSUM") as ps,
            tc.tile_pool(name="ps3", bufs=4, space="PSUM") as ps3,
        ):
            eng = lambda name: getattr(nc, name)
            # DMA issue cost ~ 0.34ns/byte-per-partition on top of a ~500ns
            # queue pitch, so wide tiles lead their queues. Plan: SP carries
            # w, ct, then the tiny e table; Pool (SWDGE) carries ctk then
            # xt2; ACT carries only xt1.
            w_sb = sb.tile([128, 192], BF16, tag="w_sb")
            eng(w_eng).dma_start(w_sb[:], wcat[:])
            ct_sb = sb.tile([32, 1024], BF16, tag="ct_sb")
            eng(e_eng).dma_start(ct_sb[:], ct[:])
            ctk_sb = sb.tile([32, 1024], BF16, tag="ctk_sb")
            eng(ct_eng).dma_start(ctk_sb[:], ctk[:])
            e_sb = sb.tile([D, 95], BF16, tag="e_sb")
            eng(e_eng).dma_start(e_sb[:], ecat[:])
            xtb = sb.tile([RG, B], BF16, tag="xtb")
            eng(x2_eng).dma_start(xtb[:], xt[RG : 2 * RG, :])
            # ACT carries only xta: its act-table load (1283ns engine) blocks
            # the ACT queue head, and the xt tiles are the late-needed inputs
            xta = sb.tile([RG, B], BF16, tag="xta")
            eng(x1_eng).dma_start(xta[:], xt[0:RG, :])

            # --- S-step: ONE matmul (K=128 H-rows). kpsum (cols 160:165)
            # and an M'-chunk (cols 168:424, partitions 0:5) share the spsum
            # bank to free PSUM banks for opsum. ---
            spsum = ps.tile([32, 424], F32, tag="spsum")
            nc.tensor.matmul(
                spsum[:, 0:160], w_sb[:, 160:192], w_sb[:, 0:160], start=True, stop=True
            )
            ssb = sb.tile([32, 160], BF16, tag="ssb")
            nc.vector.tensor_copy(ssb[:], spsum[:, 0:160])

            # --- K-step: K^T[r',d] = c * sum_q C[q] slab.T @ S_q^T ---
            for q in range(32):
                nc.tensor.matmul(
                    spsum[:, 160:165],
                    ctk_sb[:, 32 * q : 32 * (q + 1)],
                    ssb[:, q : 160 : 32],
                    start=(q == 0),
                    stop=(q == 31),
                )
            ksb = sb.tile([32, D], BF16, tag="ksb")
            nc.vector.tensor_copy(ksb[:], spsum[:, 160:165])

            # --- M'-step: M'[d,(p,m)] = sum_r' K^T[r',d] * CT[r',(p,m)],
            # in 3 chunks (512 + 256 + 256 cols) so the PSUM->SBUF copies
            # pipeline across DVE and ACT ---
            mp0 = ps.tile([D, 512], F32, tag="mp0")
            msb = sb.tile([D, 1024], BF16, tag="msb")
            # group B's bank also hosts the last M' chunk (B runs second, so
            # its write-after-read wait on the mpd copy costs little)
            mob_t = ps.tile([RG, 288], F32, tag="mob")
            mob = mob_t[:, 0:MV]
            mpc = spsum[0:D, 168:424]
            mpd = mob_t[0:D, 32:288]
            nc.tensor.matmul(
                mp0[:], ksb[:], ct_sb[:, 0:512], start=True, stop=True
            )
            # tail chunks as two 256-col matmuls into spare bank space;
            # mpd (in group B's bank) goes first so its ACT copy — which
            # group B's writes must wait for — dispatches earliest
            nc.tensor.matmul(
                mpd, ksb[:], ct_sb[:, 768:1024], start=True, stop=True
            )
            nc.tensor.matmul(
                mpc, ksb[:], ct_sb[:, 512:768], start=True, stop=True
            )
            nc.vector.tensor_copy(msb[:, 0:512], mp0[:])
            nc.scalar.copy(msb[:, 768:1024], mpd)
            nc.scalar.copy(msb[:, 512:768], mpc)

            # --- repack M'[d,(p,m)] -> two 80-row group tiles via one-hot
            # matmuls: each matmul scatters ALL FIVE d-rows of one p (rows
            # 16d+q and 64+q) — 32 matmuls total. Group A (p<16) depends
            # only on the first msb chunk; its copy overlaps group B. ---
            # group B (p>=16) runs FIRST: its msb chunks come from the ACT
            # copies whose sems land earliest, and its mb copy (ACT) then
            # overlaps group A's one-hots. mpd-backed columns (p 24..31)
            # lead within the group.
            bseq = list(range(24, 32)) + list(range(16, 24))
            for i, p in enumerate(bseq):
                q = p - 16
                nc.tensor.matmul(
                    mob,
                    e_sb[:, 15 - q : 95 - q],
                    msb[:, 32 * p : 32 * p + 32],
                    start=(i == 0),
                    stop=(i == 15),
                )
            mb = sb.tile([RG, MV], BF16, tag="mb")
            nc.scalar.copy(mb[:], mob)
            moa_t = ps.tile([RG, MV], F32, tag="moa")
            moa = moa_t[:]
            for p in range(16):
                nc.tensor.matmul(
                    moa,
                    e_sb[:, 15 - p : 95 - p],
                    msb[:, 32 * p : 32 * p + 32],
                    start=(p == 0),
                    stop=(p == 15),
                )
            ma = sb.tile([RG, MV], BF16, tag="ma")
            nc.vector.tensor_copy(ma[:], moa)

            # --- final: full batch, 4 row-blocks of 128. The group-B halves
            # open each block (mb is copied while group A's one-hots still
            # run); the group-A halves close as soon as the ma copy lands. ---
            osb = sb.tile([128, 4 * MV], F32, tag="osb")
            opsums = []
            for t in range(4):
                opsum = ps3.tile([128, MV], F32, tag="opsum")
                opsums.append(opsum)
                nc.tensor.matmul(
                    opsum[:],
                    xtb[:, 128 * t : 128 * (t + 1)],
                    mb[:],
                    start=True,
                    stop=False,
                )
            for t in range(4):
                nc.tensor.matmul(
                    opsums[t][:],
                    xta[:, 128 * t : 128 * (t + 1)],
                    ma[:],
                    start=False,
                    stop=True,
                )
                if t % 2 == 0:
                    nc.vector.tensor_copy(osb[:, MV * t : MV * (t + 1)], opsums[t][:])
                else:
                    nc.scalar.copy(osb[:, MV * t : MV * (t + 1)], opsums[t][:])
            eng(out_eng).dma_start(out[:], osb[:])

    nc.finalize()
    return nc


def make_in_maps(x_mv: np.ndarray, W_in: np.ndarray, W_out: np.ndarray) -> list:
    """Host-side layout prep: per-core input dicts (layout transforms only)."""
    x_mv = np.asarray(x_mv)
    W_in = np.asarray(W_in)
    W_out = np.asarray(W_out)

    # xt rows: per p-half, (16d+q, 64+q) packing with q = p%16
    x3 = x_mv.reshape(B, D, MV).astype(np.float32)
    halves = []
    for h in range(2):
        sl = x3[:, :, 16 * h : 16 * (h + 1)]
        halves.append(
            np.concatenate(
                [sl[:, 0:4, :].transpose(1, 2, 0).reshape(64, B), sl[:, 4, :].T],
                axis=0,
            )
        )
    xt = np.ascontiguousarray(np.concatenate(halves, axis=0)).astype(bfloat16)
    wcat = np.concatenate(
        [
            W_in.reshape(H, D * MV).astype(np.float32),
            W_out.reshape(H, MV).astype(np.float32),
        ],
        axis=1,
    ).astype(bfloat16)
    return [
        {
            "wcat": np.ascontiguousarray(wcat[128 * c : 128 * (c + 1)]),
            "ct": CT_BF,
            "ctk": CTK_BF,
            "ecat": ECAT_BF,
            "xt": xt,
        }
        for c in range(N_CORES)
    ]


_NC_CACHE: list = []


def kernel(x_mv: np.ndarray, W_in: np.ndarray, W_out: np.ndarray) -> np.ndarray:
    if not _NC_CACHE:
        _NC_CACHE.append(build_program())
    nc = _NC_CACHE[0]

    in_maps = make_in_maps(x_mv, W_in, W_out)
    try:
        res = run_bass_kernel_spmd(nc, in_maps, core_ids=list(range(N_CORES)))
    except Exception:
        # transient NRT/device hiccups have been observed; one retry
        res = run_bass_kernel_spmd(nc, in_maps, core_ids=list(range(N_CORES)))
    parts = [res.results[c]["out"] for c in range(N_CORES)]
    # device layout is [b_in_block, (t m)]; de-interleave to [t*128+b, m]
    out = np.sum(parts, axis=0).reshape(128, 4, MV).transpose(1, 0, 2)
    return np.ascontiguousarray(out, dtype=np.float32).reshape(B, 1, MV)
